# revision 26
# baseline (speedup 1.0000x reference)
"""CrossModalAttention Trainium2 kernel.

Per-core computation (data-parallel over batch, 1 sample per core):
  eeg_proj   = eeg @ W_e + b_e                  [T, U]
  image_proj = image @ W_i + b_i                [T, U]
  scores     = eeg_proj @ image_proj.T          [T, T]
  attn       = softmax(scores, axis=-1)
  att_eeg    = attn @ eeg_proj                  [T, U]
  att_img    = attn.T @ image_proj              [T, U]

Precision: projections + scores via fp32r PE matmuls (full fp32 range,
~12-bit mantissa inputs, fp32 PSUM accumulation, 4x faster than plain fp32);
softmax exp on ACT (<=2 ULP, exact per-row max subtraction); attention
weights E and AV-matmul operands in fp16. End-to-end absmax relative error
~5e-3 (set MM_DT = F32 for exact-fp32 matmuls at ~1.6x the runtime).
"""
import numpy as np
from contextlib import ExitStack

import concourse.bass as bass
import concourse.bacc as bacc
import concourse.mybir as mybir
import concourse.tile as tile
from concourse.bass_utils import run_bass_kernel_spmd
from concourse.masks import make_identity

F32 = mybir.dt.float32
F32R = mybir.dt.float32r
BF16 = mybir.dt.bfloat16
F16 = mybir.dt.float16
AX = mybir.AxisListType.X
EXP = mybir.ActivationFunctionType.Exp

B, T, DE, DI, U = 8, 2048, 512, 768, 256
NCORES = 8
TQ = T // 128          # 16 q/k tiles of 128
NK = T // 512          # 4 score chunks of 512

# ET_MODE: how attn^T tiles (lhsT for att_eeg) are produced:
#   "dma" = DMA crossbar transpose, "pe" = TensorE transpose + copy
ET_MODE = "pe"
# Matmul dtype for projections + scores: F32R (full speed, ~1.2e-4 input
# rounding) or F32 (exact, 4 cycles/row).
MM_DT = F32R
# Attention-weight / AV-rhs dtype (fp16: 4x less rounding err than bf16).
E_DT = F16


def _phase1_modality(nc, ctx, tc, pools, x_dram, w_sb, b_col, projT_sb, proj_bf):
    """Transpose x, compute projT = W.T @ x.T + b (fp32), and proj (bf16)
    via PE-transpose of projT."""
    ps, xstrip, xt, ident, identr = pools
    D = x_dram.shape[1]
    NDC = D // 128
    x = x_dram.ap()

    # 1a: transpose x [T, D] -> xT tiles [128d, T] (PE transpose, fp32 exact)
    xT = [xt.tile([128, T], MM_DT, tag="xt", name=f"xT_{dc}")
          for dc in range(NDC)]
    for s in range(T // 512):  # strips of 512 rows = 4 t-subtiles
        xs_full = xstrip.tile([128, 4, DI], F32, tag="xs", name=f"xs_{s}")
        xs = xs_full[:, :, :D]
        for tt in range(4):
            r0 = s * 512 + tt * 128
            nc.sync.dma_start(out=xs[:, tt, :], in_=x[r0:r0 + 128, :])
        for dc in range(NDC):
            pst = ps.tile([128, 512], F32, tag="ps")
            for tt in range(4):
                nc.tensor.transpose(
                    pst[:, tt * 128:(tt + 1) * 128],
                    xs[:, tt, dc * 128:(dc + 1) * 128], ident)
            nc.vector.tensor_copy(xT[dc][:, s * 512:(s + 1) * 512], pst[:])

    # 1b: projT[u, t] = sum_dc W[dc,u].T @ xT[dc] ; + b via ACT bias
    for uc in range(2):
        for nk in range(NK):
            pp = ps.tile([128, 512], F32, tag="ps")
            for dc in range(NDC):
                nc.tensor.matmul(
                    pp[:],
                    w_sb[:, dc, uc * 128:(uc + 1) * 128],
                    xT[dc][:, nk * 512:(nk + 1) * 512],
                    start=(dc == 0), stop=(dc == NDC - 1))
            nc.scalar.add(projT_sb[:, uc, nk * 512:(nk + 1) * 512], pp[:],
                          add=b_col[:, uc:uc + 1])


def _phase1c_proj(nc, ps, identr, projT_sb, proj_bf, mod):
    # proj[t, u] (E_DT) = transpose(projT)
    for tt in range(TQ):
        pq = ps.tile([128, 512], MM_DT, tag="ps", name=f"pq_{mod}_{tt}")
        for uc in range(2):
            nc.tensor.transpose(
                pq[:, uc * 128:(uc + 1) * 128],
                projT_sb[:, uc, tt * 128:(tt + 1) * 128], identr)
        nc.vector.tensor_copy(proj_bf[:, tt, :], pq[:, :U])


def build(et_mode=ET_MODE):
    nc = bacc.Bacc("TRN2", target_bir_lowering=False, debug=False,
                   num_devices=NCORES)
    eeg = nc.dram_tensor("eeg", (T, DE), F32, kind="ExternalInput")
    image = nc.dram_tensor("image", (T, DI), F32, kind="ExternalInput")
    W_e = nc.dram_tensor("W_e", (DE, U), F32, kind="ExternalInput")
    b_e = nc.dram_tensor("b_e", (U,), F32, kind="ExternalInput")
    W_i = nc.dram_tensor("W_i", (DI, U), F32, kind="ExternalInput")
    b_i = nc.dram_tensor("b_i", (U,), F32, kind="ExternalInput")
    att_e = nc.dram_tensor("att_e", (T, U), F32, kind="ExternalOutput")
    att_i = nc.dram_tensor("att_i", (T, U), F32, kind="ExternalOutput")

    with ExitStack() as ctx:
        tc = ctx.enter_context(tile.TileContext(nc))
        const = ctx.enter_context(tc.tile_pool(name="const", bufs=1))
        persist = ctx.enter_context(tc.tile_pool(name="persist", bufs=1))
        xstrip = ctx.enter_context(tc.tile_pool(name="xstrip", bufs=2))
        xt = ctx.enter_context(tc.tile_pool(name="xt", bufs=6))
        ps = ctx.enter_context(tc.tile_pool(name="ps", bufs=6, space="PSUM"))
        psb = ctx.enter_context(tc.tile_pool(name="psb", bufs=2, space="PSUM"))
        small = ctx.enter_context(tc.tile_pool(name="small", bufs=4))
        etp = ctx.enter_context(tc.tile_pool(name="etp", bufs=24))
        outp = ctx.enter_context(tc.tile_pool(name="outp", bufs=2))

        ident = const.tile([128, 128], F32)
        make_identity(nc, ident[:])
        identr = ident
        if MM_DT == F32R:
            identr = const.tile([128, 128], F32R)
            nc.vector.tensor_copy(identr[:], ident[:])
        identb = None
        if et_mode == "pe":
            identb = const.tile([128, 128], E_DT)
            make_identity(nc, identb[:])

        if MM_DT == F32R:
            # stage fp32 W in the (large, recycled) xstrip pool, round into
            # const-pool F32R tiles
            w_e_st = xstrip.tile([128, 4, DI], F32, tag="xs", name="w_e_st")
            nc.sync.dma_start(out=w_e_st[:, :DE // 128, :U],
                              in_=W_e.ap().rearrange("(c p) u -> p c u", p=128))
            w_e_sb = const.tile([128, DE // 128, U], MM_DT)
            nc.vector.tensor_copy(w_e_sb[:], w_e_st[:, :DE // 128, :U])
            w_i_st = xstrip.tile([128, 4, DI], F32, tag="xs", name="w_i_st")
            w_i_view = w_i_st[:].rearrange("p a b -> p (a b)")[:, :DI * 2].rearrange(
                "p (c u) -> p c u", u=U)
            nc.sync.dma_start(out=w_i_view,
                              in_=W_i.ap().rearrange("(c p) u -> p c u", p=128))
            w_i_sb = const.tile([128, DI // 128, U], MM_DT)
            nc.vector.tensor_copy(w_i_sb[:], w_i_view)
        else:
            w_e_sb = const.tile([128, DE // 128, U], F32)
            nc.sync.dma_start(out=w_e_sb[:], in_=W_e.ap().rearrange(
                "(c p) u -> p c u", p=128))
            w_i_sb = const.tile([128, DI // 128, U], F32)
            nc.sync.dma_start(out=w_i_sb[:], in_=W_i.ap().rearrange(
                "(c p) u -> p c u", p=128))
        be_col = const.tile([128, 2], F32)
        bi_col = const.tile([128, 2], F32)
        for c in range(2):
            nc.sync.dma_start(out=be_col[:, c:c + 1], in_=b_e.ap()[
                c * 128:(c + 1) * 128].rearrange("(p o) -> p o", o=1))
            nc.sync.dma_start(out=bi_col[:, c:c + 1], in_=b_i.ap()[
                c * 128:(c + 1) * 128].rearrange("(p o) -> p o", o=1))

        projTe = persist.tile([128, 2, T], MM_DT, tag="projTe")
        projTi = persist.tile([128, 2, T], MM_DT, tag="projTi")
        proj_e_bf = persist.tile([128, TQ, U], E_DT, tag="proj_e_bf")
        proj_i_bf = persist.tile([128, TQ, U], E_DT, tag="proj_i_bf")
        E = persist.tile([128, TQ, T], E_DT, tag="E")
        rZ = persist.tile([128, TQ], F32, tag="rZ")

        pools = (ps, xstrip, xt, ident, identr)
        _phase1_modality(nc, ctx, tc, pools, eeg, w_e_sb, be_col, projTe,
                         proj_e_bf)
        _phase1c_proj(nc, ps, identr, projTe, proj_e_bf, "e")
        _phase1_modality(nc, ctx, tc, pools, image, w_i_sb, bi_col, projTi,
                         proj_i_bf)
        _phase1c_proj(nc, ps, identr, projTi, proj_i_bf, "i")

        # Phase 2 (software-pipelined on PE): emit scores/softmax(qt),
        # then the att_eeg block of qt-1 so PE never stalls on exp(qt).
        def emit_scores(qt):
            cm = small.tile([128, 4], F32, tag="cm", name=f"cm_{qt}")
            s_chunks = []
            for nk in range(NK):
                s = ps.tile([128, 512], F32, tag="ps", name=f"s_{qt}_{nk}")
                s_chunks.append(s)
                for uc in range(2):
                    nc.tensor.matmul(
                        s[:],
                        projTe[:, uc, qt * 128:(qt + 1) * 128],
                        projTi[:, uc, nk * 512:(nk + 1) * 512],
                        start=(uc == 0), stop=(uc == 1))
                nc.vector.reduce_max(cm[:, nk:nk + 1], s[:], axis=AX)
            negmax = small.tile([128, 1], F32, tag="negmax", name=f"nm_{qt}")
            nc.vector.tensor_reduce(negmax[:], cm[:], axis=AX,
                                    op=mybir.AluOpType.max, negate=True)
            zp = small.tile([128, 4], F32, tag="zp", name=f"zp_{qt}")
            for nk in range(NK):
                nc.scalar.activation(
                    E[:, qt, nk * 512:(nk + 1) * 512], s_chunks[nk][:], EXP,
                    bias=negmax[:], scale=1.0, accum_out=zp[:, nk:nk + 1])
            zrow = small.tile([128, 1], F32, tag="zrow", name=f"zr_{qt}")
            nc.vector.reduce_sum(zrow[:], zp[:], axis=AX)
            nc.vector.reciprocal(rZ[:, qt:qt + 1], zrow[:])
            nc.vector.tensor_scalar_mul(
                proj_i_bf[:, qt, :], proj_i_bf[:, qt, :], rZ[:, qt:qt + 1])

        def emit_av_eeg(qt):
            # att_eeg[qt] = (1/Z) * sum_k E^T[k, qt] @ eeg_proj[k]
            if et_mode == "dma":
                etts = []
                for kc in range(TQ):
                    ett = etp.tile([128, 128], E_DT, tag="ett",
                                   name=f"ett_{qt}_{kc}")
                    nc.sync.dma_start_transpose(
                        ett[:], E[:, qt, kc * 128:(kc + 1) * 128])
                    etts.append(ett[:])
            else:
                etts = []
                for kg in range(4):
                    pet = psb.tile([128, 512], E_DT, tag="pet",
                                   name=f"pet_{qt}_{kg}")
                    for j in range(4):
                        nc.tensor.transpose(
                            pet[:, j * 128:(j + 1) * 128],
                            E[:, qt, (kg * 4 + j) * 128:(kg * 4 + j + 1) * 128],
                            identb[:])
                    ettg = etp.tile([128, 512], E_DT, tag="ett", bufs=8,
                                    name=f"ettg_{qt}_{kg}")
                    if kg % 2 == 0:
                        nc.vector.tensor_copy(ettg[:], pet[:])
                    else:
                        nc.scalar.copy(ettg[:], pet[:])
                    etts.extend(ettg[:, j * 128:(j + 1) * 128]
                                for j in range(4))
            pav = ps.tile([128, 512], F32, tag="ps", name=f"pav_{qt}")
            for kc in range(TQ):
                nc.tensor.matmul(pav[:, :U], etts[kc], proj_e_bf[:, kc, :],
                                 start=(kc == 0), stop=(kc == TQ - 1))
            oe = outp.tile([128, U], F32, tag="out", name=f"oe_{qt}")
            nc.scalar.activation(oe[:], pav[:, :U],
                                 mybir.ActivationFunctionType.Copy,
                                 scale=rZ[:, qt:qt + 1])
            nc.sync.dma_start(out=att_e.ap()[qt * 128:(qt + 1) * 128, :],
                              in_=oe[:])

        for qt in range(TQ):
            emit_scores(qt)
            if qt >= 1:
                emit_av_eeg(qt - 1)
        emit_av_eeg(TQ - 1)

        # Phase 3: att_img[kt] = sum_q E[q, kt].T(as lhsT) @ (image_proj/Z)[q]
        for kt in range(TQ):
            pav = ps.tile([128, 512], F32, tag="ps")
            for qc in range(TQ):
                nc.tensor.matmul(
                    pav[:, :U], E[:, qc, kt * 128:(kt + 1) * 128],
                    proj_i_bf[:, qc, :],
                    start=(qc == 0), stop=(qc == TQ - 1))
            oi = outp.tile([128, U], F32, tag="out")
            nc.scalar.copy(oi[:], pav[:, :U])
            nc.sync.dma_start(out=att_i.ap()[kt * 128:(kt + 1) * 128, :],
                              in_=oi[:])

    nc.finalize()
    return nc


_NC_CACHE = {}


def kernel(eeg, image, W_e, b_e, W_i, b_i):
    key = ET_MODE
    if key not in _NC_CACHE:
        _NC_CACHE[key] = build(key)
    nc = _NC_CACHE[key]
    eeg = np.ascontiguousarray(eeg, dtype=np.float32)
    image = np.ascontiguousarray(image, dtype=np.float32)
    in_maps = [{
        "eeg": eeg[b], "image": image[b],
        "W_e": np.asarray(W_e, np.float32), "b_e": np.asarray(b_e, np.float32),
        "W_i": np.asarray(W_i, np.float32), "b_i": np.asarray(b_i, np.float32),
    } for b in range(B)]
    res = run_bass_kernel_spmd(nc, in_maps, list(range(NCORES)))
    att_e = np.stack([np.asarray(r["att_e"]) for r in res.results])
    att_i = np.stack([np.asarray(r["att_i"]) for r in res.results])
    return att_e, att_i
